# revision 15
# baseline (speedup 1.0000x reference)
"""Trainium2 Bass kernel for nn_CoAttention_TextDNS.

Math: both additive co-attention blocks have scores of the form
    score[l, m] = f(l) + g(m) + const
followed by softmax over the last axis, so the row-dependent terms cancel
(softmax shift invariance) and the attention weights are identical for every
row l:
    att_dns[b]  = broadcast_rows( softmax(tanh(dns[b]  @ W_d1.T) @ wb) @ dns[b] )
    att_text[b] = broadcast_rows( softmax(tanh(text[b] @ W_t2.T) @ wd) @ text[b] )
with wb = w_att1[H:], wd = w_att2[H:].  W_t1/b_t1/W_d2/b_d2/wa/wc/b_att1/
b_att2 do not affect the output.

Sharding: data-parallel over batch, one batch element per NeuronCore (B=8).

The device computes only the score rows u1 = tanh(dns@W_d1.T)@wb and
u2 = tanh(text@W_t2.T)@wd (the DMA- and FLOP-heavy part); the host finishes
with softmax, the tiny p@X row-sum in fp32, and the row broadcast.

Device layout: scores are built as out[o, l] = (W.T chunk).T @ (X.T chunk)
accumulated over h-chunks, so the w-projection contracts over PSUM
partitions (o) and becomes a 1-row accumulating PE matmul against the
tanh tiles — no DVE reduction, no transposes.  Everything the device
touches is bf16 (halves DMA + full PE rate); PSUM accumulation is fp32.
"""

import numpy as np

B, L, M, H = 8, 256, 128, 768
SX = 16.0    # activation fp8 pre-scale (N(0,1) -> e4m3 normal range)
SW = 4096.0  # weight fp8 pre-scale (U(+-1/sqrt(H)) -> e4m3 normal range)
HC = H // 128  # 6 contraction chunks of 128
OC = H // 128  # 6 output-column tiles of 128


def _build_module(reps=1):
    """Build the per-core module. reps>1 wraps the main pipeline in an
    on-device hardware loop — used only for wall-clock benchmarking (the
    axon dispatch RTT swamps a single ~10 us execution)."""
    import concourse.tile as tile
    from concourse import bacc, mybir
    from contextlib import nullcontext

    f32 = mybir.dt.float32
    bf16 = mybir.dt.bfloat16

    nc = bacc.Bacc("TRN2", target_bir_lowering=False, debug=False)

    u8 = mybir.dt.uint8
    # Score operands ship as fp8 (float8e3 == e3m4) bytes declared uint8 so
    # the PJRT staging path never sees the e3m4 dtype; on-device APs bitcast.
    dnst = nc.dram_tensor("dnst", [128, HC * M], u8, kind="ExternalInput").ap()
    textt = nc.dram_tensor("textt", [128, HC * L], u8, kind="ExternalInput").ap()
    wd1t = nc.dram_tensor("wd1t", [128, OC * HC * 128], u8, kind="ExternalInput").ap()
    wt2t = nc.dram_tensor("wt2t", [128, OC * HC * 128], u8, kind="ExternalInput").ap()
    wcol = nc.dram_tensor("wcol", [128, 2 * OC], bf16, kind="ExternalInput").ap()
    u_out = nc.dram_tensor("u", [1, M + L], f32, kind="ExternalOutput").ap()

    Tanh = mybir.ActivationFunctionType.Tanh

    with tile.TileContext(nc) as tc:
        with (
            tc.tile_pool(name="ins", bufs=1) as ins,
            tc.tile_pool(name="work", bufs=1) as work,
            tc.tile_pool(name="d1ps", bufs=3, space="PSUM") as d1ps,
            tc.tile_pool(name="t2ps", bufs=3, space="PSUM") as t2ps,
        ):
            # Junk tile for PE warm-up (HAM clock gate releases after ~3.4us
            # of sustained activity; the cost model's p-state ramp likewise).
            # Warm PSUM comes from the d1 pool so no extra bank is burned.
            zjunk = ins.tile([128, 128], bf16, tag="zjunk")
            nc.vector.memset(zjunk, 0.0)
            warm_ps = t2ps.tile([128, 128], f32, tag="psL")
            for _ in range(18):
                nc.tensor.matmul(warm_ps, zjunk, zjunk, start=True, stop=True)

            loop = tc.For_i(0, reps, 1) if reps > 1 else nullcontext()
            with loop:
                _pipeline_body(nc, tc, ins, work, d1ps, t2ps, mybir,
                               dnst, textt, wd1t, wt2t, wcol, u_out,
                               Tanh, f32, bf16, u8)

    nc.compile()
    return nc


def _pipeline_body(nc, tc, ins, work, d1ps, t2ps, mybir,
                   dnst, textt, wd1t, wt2t, wcol, u_out, Tanh, f32, bf16, u8):
    f8 = mybir.dt.float8e4
    DoubleRow = mybir.MatmulPerfMode.DoubleRow

    # ---- input DMAs (issue order = stream order on the sync ring) -------
    # Big t2 weight tile first; the small tensors (textt/wcol/dnst) hide in
    # the following tiles' HWDGE+DGE latency.  wd1 arrives in o-tile pairs
    # matched to the PSUM pair grouping.  8 input DMAs keeps the global
    # HWDGE chain (~630ns each) just ahead of the transfer stream.
    wt2_sb = ins.tile([128, OC, HC, 128], u8, tag="wt2")
    wt2_r = wt2t.rearrange("p (co ch o) -> p co ch o", co=OC, ch=HC)
    nc.sync.dma_start(out=wt2_sb[:, 0:3], in_=wt2_r[:, 0:3])
    textt_sb = ins.tile([128, HC, L], u8, tag="textt")
    nc.sync.dma_start(out=textt_sb, in_=textt.rearrange("p (c l) -> p c l", c=HC))
    wcol_sb = ins.tile([128, 2 * OC], bf16, tag="wcol")
    nc.scalar.dma_start(out=wcol_sb, in_=wcol)
    nc.sync.dma_start(out=wt2_sb[:, 3:6], in_=wt2_r[:, 3:6])
    dnst_sb = ins.tile([128, HC, M], u8, tag="dnst")
    nc.sync.dma_start(out=dnst_sb, in_=dnst.rearrange("p (c m) -> p c m", c=HC))
    wd1_sb = ins.tile([128, OC, HC, 128], u8, tag="wd1")
    wd1_r = wd1t.rearrange("p (co ch o) -> p co ch o", co=OC, ch=HC)
    for g in range(3):
        nc.sync.dma_start(
            out=wd1_sb[:, 2 * g : 2 * g + 2], in_=wd1_r[:, 2 * g : 2 * g + 2]
        )

    usb = work.tile([1, M + L], f32, tag="usb")
    act2 = work.tile([128, OC, L], bf16, tag="act2")
    act1 = work.tile([128, OC, M], bf16, tag="act1")

    def mm_group(ps_pool, w_sb, x_sb, n, act, cos):
        """DoubleRow score matmuls for o-tiles `cos` + one tanh over all."""
        ps = ps_pool.tile([128, len(cos), n], f32, tag="psL" if n == L else "psM")
        for i, co in enumerate(cos):
            for cp in range(HC // 2):
                nc.tensor.matmul(
                    ps[:, i, :],
                    w_sb[:, co, 2 * cp : 2 * cp + 2, :].bitcast(f8),
                    x_sb[:, 2 * cp : 2 * cp + 2, :].bitcast(f8),
                    start=(cp == 0), stop=(cp == HC // 2 - 1),
                    perf_mode=DoubleRow,
                )
        # PSUM holds (SX*SW)*scores; fold the dequant into the tanh.
        nc.scalar.activation(
            act[:, cos[0] : cos[0] + len(cos), :], ps, Tanh, scale=1.0 / (SX * SW)
        )

    def proj(act, wcol_off, u_ps, co):
        nc.tensor.matmul(
            u_ps, wcol_sb[:, wcol_off + co : wcol_off + co + 1],
            act[:, co, :], start=(co == 0), stop=(co == OC - 1),
        )

    def finish(u_ps, u_sl):
        nc.vector.tensor_copy(out=usb[:, u_sl], in_=u_ps)
        # Out-DMAs ride the ACT ring: they'd block the SP ring's input
        # stream for the next For_i iteration while waiting on the copy.
        nc.scalar.dma_start(out=u_out[:, u_sl], in_=usb[:, u_sl])

    # t2 pairs with lagging projs; d1 mm-groups slot into the PE queue where
    # a proj would otherwise stall on the ACT chain.  The u-row accumulators
    # recycle score banks whose tanh is already their own data dependency,
    # so the WAR is free.
    t2mm = lambda cos: mm_group(t2ps, wt2_sb, textt_sb, L, act2, cos)
    d1mm = lambda cos: mm_group(d1ps, wd1_sb, dnst_sb, M, act1, cos)

    t2mm((0, 1))
    t2mm((2, 3))
    u2_ps = t2ps.tile([1, L], f32, tag="psL")
    p2 = lambda co: proj(act2, OC, u2_ps, co)
    t2mm((4, 5))
    p2(0)
    p2(1)
    d1mm((0, 1))
    p2(2)
    p2(3)
    d1mm((2, 3))
    p2(4)
    p2(5)
    finish(u2_ps, slice(M, M + L))
    d1mm((4, 5))
    u1_ps = d1ps.tile([1, M], f32, tag="psM")
    p1 = lambda co: proj(act1, 0, u1_ps, co)
    for co in range(OC):
        p1(co)
    finish(u1_ps, slice(0, M))


_NC_CACHE = {}


def _get_module(reps=1):
    if reps not in _NC_CACHE:
        _NC_CACHE[reps] = _build_module(reps)
    return _NC_CACHE[reps]


def _bf16(x):
    import ml_dtypes

    return np.ascontiguousarray(np.asarray(x, np.float32)).astype(ml_dtypes.bfloat16)


def _f8(x, scale):
    """fp32 -> scaled float8_e3m4, shipped as raw uint8 bytes."""
    import ml_dtypes

    q = (np.ascontiguousarray(np.asarray(x, np.float32)) * scale).astype(
        ml_dtypes.float8_e4m3
    )
    return q.view(np.uint8)


def _chunked_T(x, inner):
    """[R, H] -> [128, HC*inner] fp8 with [p, c*inner + r] = x[r, c*128 + p]."""
    r = x.shape[0]
    assert x.shape == (r, H) and r == inner
    return _f8(
        x.T.reshape(HC, 128, inner).transpose(1, 0, 2).reshape(128, HC * inner), SX
    )


def _w_tiles(w):
    """[H, H] -> [128, OC*HC*128] fp8 with [p, ((co*HC)+ch)*128 + o] =
    w[co*128 + o, ch*128 + p]."""
    t = w.reshape(OC, 128, HC, 128).transpose(3, 0, 2, 1)  # [p, co, ch, o]
    return _f8(t.reshape(128, OC * HC * 128), SW)


def _make_in_maps(kernel_inputs):
    text = np.asarray(kernel_inputs["text_features"], np.float32)
    dns = np.asarray(kernel_inputs["dns_features"], np.float32)
    W_d1 = np.asarray(kernel_inputs["W_d1"], np.float32)
    W_t2 = np.asarray(kernel_inputs["W_t2"], np.float32)
    wb = np.asarray(kernel_inputs["w_att1"], np.float32)[H:]
    wd = np.asarray(kernel_inputs["w_att2"], np.float32)[H:]

    wd1t = _w_tiles(W_d1)
    wt2t = _w_tiles(W_t2)
    wcol = _bf16(
        np.concatenate(
            [wb.reshape(OC, 128).T, wd.reshape(OC, 128).T], axis=1
        )  # [128, 2*OC]
    )

    in_maps = []
    for b in range(B):
        in_maps.append(
            {
                "dnst": _chunked_T(dns[b], M),
                "textt": _chunked_T(text[b], L),
                "wd1t": wd1t,
                "wt2t": wt2t,
                "wcol": wcol,
            }
        )
    return in_maps


def _run_device(kernel_inputs):
    from concourse.bass_utils import run_bass_kernel_spmd

    in_maps = _make_in_maps(kernel_inputs)
    nc = _get_module()
    return run_bass_kernel_spmd(nc, in_maps, list(range(B)))


def _softmax(u):
    e = np.exp(u - u.max())
    return e / e.sum()


def kernel(**inputs):
    res = _run_device(inputs)
    text = np.asarray(inputs["text_features"], np.float32)
    dns = np.asarray(inputs["dns_features"], np.float32)
    att_text = np.empty((B, L, H), np.float32)
    att_dns = np.empty((B, L, H), np.float32)
    for b in range(B):
        u = np.asarray(res.results[b]["u"], np.float32).reshape(M + L)
        v1 = _softmax(u[:M]) @ dns[b]  # (H,)
        v2 = _softmax(u[M:]) @ text[b]
        att_dns[b] = v1[None, :]
        att_text[b] = v2[None, :]
    return att_text, att_dns


# revision 16
# speedup vs baseline: 1.1802x; 1.1802x over previous
"""Trainium2 Bass kernel for nn_CoAttention_TextDNS.

Math: both additive co-attention blocks have scores of the form
    score[l, m] = f(l) + g(m) + const
followed by softmax over the last axis, so the row-dependent terms cancel
(softmax shift invariance) and the attention weights are identical for every
row l:
    att_dns[b]  = broadcast_rows( softmax(tanh(dns[b]  @ W_d1.T) @ wb) @ dns[b] )
    att_text[b] = broadcast_rows( softmax(tanh(text[b] @ W_t2.T) @ wd) @ text[b] )
with wb = w_att1[H:], wd = w_att2[H:].  W_t1/b_t1/W_d2/b_d2/wa/wc/b_att1/
b_att2 do not affect the output.

Sharding: data-parallel over batch, one batch element per NeuronCore (B=8).

The device computes only the score rows u1 = tanh(dns@W_d1.T)@wb and
u2 = tanh(text@W_t2.T)@wd (the DMA- and FLOP-heavy part); the host finishes
with softmax, the tiny p@X row-sum in fp32, and the row broadcast.

Device layout: scores are built as out[o, l] = (W.T chunk).T @ (X.T chunk)
accumulated over h-chunks, so the w-projection contracts over PSUM
partitions (o) and becomes a 1-row accumulating PE matmul against the
tanh tiles — no DVE reduction, no transposes.  Everything the device
touches is bf16 (halves DMA + full PE rate); PSUM accumulation is fp32.
"""

import numpy as np

B, L, M, H = 8, 256, 128, 768
SX = 16.0    # activation fp8 pre-scale (N(0,1) -> e4m3 normal range)
SW = 4096.0  # weight fp8 pre-scale (U(+-1/sqrt(H)) -> e4m3 normal range)
HC = H // 128  # 6 contraction chunks of 128
OC = H // 128  # 6 output-column tiles of 128


def _build_module(reps=1):
    """Build the per-core module. reps>1 wraps the main pipeline in an
    on-device hardware loop — used only for wall-clock benchmarking (the
    axon dispatch RTT swamps a single ~10 us execution)."""
    import concourse.tile as tile
    from concourse import bacc, mybir
    from contextlib import nullcontext

    f32 = mybir.dt.float32
    bf16 = mybir.dt.bfloat16

    nc = bacc.Bacc("TRN2", target_bir_lowering=False, debug=False)

    u8 = mybir.dt.uint8
    # Score operands ship as fp8 (float8e3 == e3m4) bytes declared uint8 so
    # the PJRT staging path never sees the e3m4 dtype; on-device APs bitcast.
    dnst = nc.dram_tensor("dnst", [128, HC * M], u8, kind="ExternalInput").ap()
    textt = nc.dram_tensor("textt", [128, HC * L], u8, kind="ExternalInput").ap()
    wd1t = nc.dram_tensor("wd1t", [128, OC * HC * 128], u8, kind="ExternalInput").ap()
    wt2t = nc.dram_tensor("wt2t", [128, OC * HC * 128], u8, kind="ExternalInput").ap()
    wcol = nc.dram_tensor("wcol", [128, 2 * OC], bf16, kind="ExternalInput").ap()
    u_out = nc.dram_tensor("u", [1, M + L], f32, kind="ExternalOutput").ap()

    Tanh = mybir.ActivationFunctionType.Tanh

    with tile.TileContext(nc) as tc:
        with (
            tc.tile_pool(name="ins", bufs=1) as ins,
            tc.tile_pool(name="work", bufs=1) as work,
            tc.tile_pool(name="d1ps", bufs=3, space="PSUM") as d1ps,
            tc.tile_pool(name="t2ps", bufs=3, space="PSUM") as t2ps,
        ):
            # Junk tile for PE warm-up (HAM clock gate releases after ~3.4us
            # of sustained activity; the cost model's p-state ramp likewise).
            # Warm PSUM comes from the d1 pool so no extra bank is burned.
            zjunk = ins.tile([128, 128], bf16, tag="zjunk")
            nc.vector.memset(zjunk, 0.0)
            warm_ps = t2ps.tile([128, 128], f32, tag="psL")
            for _ in range(18):
                nc.tensor.matmul(warm_ps, zjunk, zjunk, start=True, stop=True)

            loop = tc.For_i(0, reps, 1) if reps > 1 else nullcontext()
            with loop:
                _pipeline_body(nc, tc, ins, work, d1ps, t2ps, mybir,
                               dnst, textt, wd1t, wt2t, wcol, u_out,
                               Tanh, f32, bf16, u8)

    nc.compile()
    return nc


def _pipeline_body(nc, tc, ins, work, d1ps, t2ps, mybir,
                   dnst, textt, wd1t, wt2t, wcol, u_out, Tanh, f32, bf16, u8):
    f8 = mybir.dt.float8e4
    DoubleRow = mybir.MatmulPerfMode.DoubleRow

    # ---- input DMAs (issue order = stream order on the sync ring) -------
    # Big t2 weight tile first; the small tensors (textt/wcol/dnst) hide in
    # the following tiles' HWDGE+DGE latency.  wd1 arrives in o-tile pairs
    # matched to the PSUM pair grouping.  8 input DMAs keeps the global
    # HWDGE chain (~630ns each) just ahead of the transfer stream.
    wt2_sb = ins.tile([128, OC, HC, 128], u8, tag="wt2")
    wt2_r = wt2t.rearrange("p (co ch o) -> p co ch o", co=OC, ch=HC)
    nc.sync.dma_start(out=wt2_sb[:, 0:3], in_=wt2_r[:, 0:3])
    textt_sb = ins.tile([128, HC, L], u8, tag="textt")
    nc.sync.dma_start(out=textt_sb, in_=textt.rearrange("p (c l) -> p c l", c=HC))
    wcol_sb = ins.tile([128, 2 * OC], bf16, tag="wcol")
    nc.scalar.dma_start(out=wcol_sb, in_=wcol)
    nc.sync.dma_start(out=wt2_sb[:, 3:6], in_=wt2_r[:, 3:6])
    dnst_sb = ins.tile([128, HC, M], u8, tag="dnst")
    nc.sync.dma_start(out=dnst_sb, in_=dnst.rearrange("p (c m) -> p c m", c=HC))
    wd1_sb = ins.tile([128, OC, HC, 128], u8, tag="wd1")
    wd1_r = wd1t.rearrange("p (co ch o) -> p co ch o", co=OC, ch=HC)
    for g in range(3):
        nc.sync.dma_start(
            out=wd1_sb[:, 2 * g : 2 * g + 2], in_=wd1_r[:, 2 * g : 2 * g + 2]
        )

    usb = work.tile([1, M + L], f32, tag="usb")
    act2 = work.tile([128, OC, L], bf16, tag="act2")
    act1 = work.tile([128, OC, M], bf16, tag="act1")

    def mm_group(ps_pool, w_sb, x_sb, n, act, cos):
        """DoubleRow score matmuls for o-tiles `cos` + one tanh over all."""
        ps = ps_pool.tile([128, len(cos), n], f32, tag="psL" if n == L else "psM")
        for i, co in enumerate(cos):
            for cp in range(HC // 2):
                nc.tensor.matmul(
                    ps[:, i, :],
                    w_sb[:, co, 2 * cp : 2 * cp + 2, :].bitcast(f8),
                    x_sb[:, 2 * cp : 2 * cp + 2, :].bitcast(f8),
                    start=(cp == 0), stop=(cp == HC // 2 - 1),
                    perf_mode=DoubleRow,
                )
        # PSUM holds (SX*SW)*scores; fold the dequant into the tanh.
        nc.scalar.activation(
            act[:, cos[0] : cos[0] + len(cos), :], ps, Tanh, scale=1.0 / (SX * SW)
        )

    def proj(act, wcol_off, u_ps, co):
        nc.tensor.matmul(
            u_ps, wcol_sb[:, wcol_off + co : wcol_off + co + 1],
            act[:, co, :], start=(co == 0), stop=(co == OC - 1),
        )

    def finish(u_ps, u_sl):
        nc.vector.tensor_copy(out=usb[:, u_sl], in_=u_ps)
        nc.sync.dma_start(out=u_out[:, u_sl], in_=usb[:, u_sl])

    # t2 pairs with lagging projs; d1 mm-groups slot into the PE queue where
    # a proj would otherwise stall on the ACT chain.  The u-row accumulators
    # recycle score banks whose tanh is already their own data dependency,
    # so the WAR is free.
    t2mm = lambda cos: mm_group(t2ps, wt2_sb, textt_sb, L, act2, cos)
    d1mm = lambda cos: mm_group(d1ps, wd1_sb, dnst_sb, M, act1, cos)

    t2mm((0, 1))
    t2mm((2, 3))
    u2_ps = t2ps.tile([1, L], f32, tag="psL")
    p2 = lambda co: proj(act2, OC, u2_ps, co)
    t2mm((4, 5))
    p2(0)
    p2(1)
    d1mm((0, 1))
    p2(2)
    p2(3)
    d1mm((2, 3))
    p2(4)
    p2(5)
    finish(u2_ps, slice(M, M + L))
    d1mm((4, 5))
    u1_ps = d1ps.tile([1, M], f32, tag="psM")
    p1 = lambda co: proj(act1, 0, u1_ps, co)
    for co in range(OC):
        p1(co)
    finish(u1_ps, slice(0, M))


_NC_CACHE = {}


def _get_module(reps=1):
    if reps not in _NC_CACHE:
        _NC_CACHE[reps] = _build_module(reps)
    return _NC_CACHE[reps]


def _bf16(x):
    import ml_dtypes

    return np.ascontiguousarray(np.asarray(x, np.float32)).astype(ml_dtypes.bfloat16)


def _f8(x, scale):
    """fp32 -> scaled float8_e3m4, shipped as raw uint8 bytes."""
    import ml_dtypes

    q = (np.ascontiguousarray(np.asarray(x, np.float32)) * scale).astype(
        ml_dtypes.float8_e4m3
    )
    return q.view(np.uint8)


def _chunked_T(x, inner):
    """[R, H] -> [128, HC*inner] fp8 with [p, c*inner + r] = x[r, c*128 + p]."""
    r = x.shape[0]
    assert x.shape == (r, H) and r == inner
    return _f8(
        x.T.reshape(HC, 128, inner).transpose(1, 0, 2).reshape(128, HC * inner), SX
    )


def _w_tiles(w):
    """[H, H] -> [128, OC*HC*128] fp8 with [p, ((co*HC)+ch)*128 + o] =
    w[co*128 + o, ch*128 + p]."""
    t = w.reshape(OC, 128, HC, 128).transpose(3, 0, 2, 1)  # [p, co, ch, o]
    return _f8(t.reshape(128, OC * HC * 128), SW)


def _make_in_maps(kernel_inputs):
    text = np.asarray(kernel_inputs["text_features"], np.float32)
    dns = np.asarray(kernel_inputs["dns_features"], np.float32)
    W_d1 = np.asarray(kernel_inputs["W_d1"], np.float32)
    W_t2 = np.asarray(kernel_inputs["W_t2"], np.float32)
    wb = np.asarray(kernel_inputs["w_att1"], np.float32)[H:]
    wd = np.asarray(kernel_inputs["w_att2"], np.float32)[H:]

    wd1t = _w_tiles(W_d1)
    wt2t = _w_tiles(W_t2)
    wcol = _bf16(
        np.concatenate(
            [wb.reshape(OC, 128).T, wd.reshape(OC, 128).T], axis=1
        )  # [128, 2*OC]
    )

    in_maps = []
    for b in range(B):
        in_maps.append(
            {
                "dnst": _chunked_T(dns[b], M),
                "textt": _chunked_T(text[b], L),
                "wd1t": wd1t,
                "wt2t": wt2t,
                "wcol": wcol,
            }
        )
    return in_maps


def _run_device(kernel_inputs):
    from concourse.bass_utils import run_bass_kernel_spmd

    in_maps = _make_in_maps(kernel_inputs)
    nc = _get_module()
    return run_bass_kernel_spmd(nc, in_maps, list(range(B)))


def _softmax(u):
    e = np.exp(u - u.max())
    return e / e.sum()


def kernel(**inputs):
    res = _run_device(inputs)
    text = np.asarray(inputs["text_features"], np.float32)
    dns = np.asarray(inputs["dns_features"], np.float32)
    att_text = np.empty((B, L, H), np.float32)
    att_dns = np.empty((B, L, H), np.float32)
    for b in range(B):
        u = np.asarray(res.results[b]["u"], np.float32).reshape(M + L)
        v1 = _softmax(u[:M]) @ dns[b]  # (H,)
        v2 = _softmax(u[M:]) @ text[b]
        att_dns[b] = v1[None, :]
        att_text[b] = v2[None, :]
    return att_text, att_dns
